# revision 2
# baseline (speedup 1.0000x reference)
"""Trainium2 Bass kernel for nn_CenterModel (Fourier-basis formulation).

Computes -sum_w max_o ( C[w]*cos(o) - S[w]*sin(o) ) where
  C[w] = mean_n cos(2*pi*dist(n)/lambda[w]) * tid[n, w]
  S[w] = mean_n sin(2*pi*dist(n)/lambda[w]) * tid[n, w]

Key restructure vs the direct method: expand the wavelength-dependent
trig in a shared Fourier basis over distance,
  cos(alpha_w d) ~= sum_k A[k,w] phi_k(d),   phi = {1, cos(2*pi*k*d/P), sin(2*pi*k*d/P)}
so the device only computes K_B = 2*KH+1 = 45 basis columns per point
(instead of 128 wavelength columns) and one accumulated matmul
  M[w, k] = sum_n tid[n, w] * phi_k(d_n)        (PSUM, fp32)
The wavelength-dependent combine C = sum_k A[k,w] M[w,k] runs on the
host from a ridge least-squares fit against the *runtime* wavelengths
(fit residual < 2e-3 for lambda >= 0.048, d <= 0.71).

Device pipeline per 128-point tile group (NT=32 tiles):
  e = dist/P once per point (Ln/Exp sqrt, /P folded into Exp bias);
  u = e * k (one broadcast TT), m = round(u) (fused magic TS, 2x),
  ds = u - m (TT), ads = |ds| (TS bitand, 2x);
  sin cols = Sin(2*pi*ds), cos cols = Sin(pi/2 - 2*pi*ads) on ScalarE;
  tid cast fp32->fp16 split DVE/ScalarE; 32 matmuls tid^T @ phi.
8 cores shard the 500000 points; host sums the per-core M.
"""

import math
import os
from contextlib import ExitStack

import numpy as np

import concourse.bacc as bacc
import concourse.bass as bass
import concourse.tile as tile
from concourse import mybir
from concourse.bass_utils import run_bass_kernel_spmd

F32 = mybir.dt.float32
F16 = mybir.dt.float16
U32 = mybir.dt.uint32
AF = mybir.ActivationFunctionType
OP = mybir.AluOpType

N_POINTS = 500000
W = 128
N_OFFSETS = 50
N_CORES = 8
PER_CORE = N_POINTS // N_CORES  # 62500
NPP = 489                       # point-tiles per core
N_PAD = NPP * 128               # 62592 padded rows per core
NT = 32                         # point-tiles per super-tile
TWO_PI = 2.0 * math.pi

MAGIC = 12582912.0  # 1.5*2**23: fl(u+MAGIC)-MAGIC == round(u) for |u| < 2**22

# ---- Fourier basis config (must match host fit exactly) ----
P_BASIS = 0.8          # extension period
KH = 22                # harmonics 1..KH
K_B = 2 * KH + 1       # ones + cos_k + sin_k = 45
DMAX_FIT = math.sqrt(0.5) + 2e-3
# envelope in which the fit is trusted (runtime inputs checked on host)
LAM_MIN_OK = 0.048
DMAX_OK = math.sqrt(0.5) + 1e-6

_TID_LOAD = os.environ.get("KERNEL_TID_LOAD", "split")  # split | dve | swdge

_cached_nc = None


def _build_program():
    nc = bacc.Bacc(
        "TRN2",
        debug=False,
        enable_asserts=False,
        target_bir_lowering=False,
        num_devices=N_CORES,
    )
    xy_d = nc.dram_tensor("xy", [N_PAD, 2], F32, kind="ExternalInput")
    tid_d = nc.dram_tensor("tid", [N_PAD, W], F32, kind="ExternalInput")
    cen_d = nc.dram_tensor("center", [2], F32, kind="ExternalInput")
    kvec_d = nc.dram_tensor("kvec", [KH], F32, kind="ExternalInput")
    out_d = nc.dram_tensor("out", [W, K_B], F32, kind="ExternalOutput")

    with tile.TileContext(nc) as tc, ExitStack() as ctx:
        consts = ctx.enter_context(tc.tile_pool(name="consts", bufs=1))
        tid32p = ctx.enter_context(tc.tile_pool(name="tid32p", bufs=3))
        tid16p = ctx.enter_context(tc.tile_pool(name="tid16p", bufs=2))
        up = ctx.enter_context(tc.tile_pool(name="up", bufs=2))
        mp = ctx.enter_context(tc.tile_pool(name="mp", bufs=2))
        dsp = ctx.enter_context(tc.tile_pool(name="dsp", bufs=2))
        adsp = ctx.enter_context(tc.tile_pool(name="adsp", bufs=2))
        trigp = ctx.enter_context(tc.tile_pool(name="trigp", bufs=2))
        psump = ctx.enter_context(tc.tile_pool(name="psump", bufs=1, space="PSUM"))

        # ---------------- constants ----------------
        # kvec (harmonic indices 1..KH) broadcast to all partitions
        kb = consts.tile([128, KH], F32)
        kv_ap = kvec_d[:]
        nc.gpsimd.dma_start(
            out=kb,
            in_=bass.AP(
                tensor=kv_ap.tensor,
                offset=kv_ap.offset,
                ap=[[0, 128]] + list(kv_ap.ap),
            ),
        )
        # center broadcast to all partitions
        cen = consts.tile([128, 2], F32)
        cen_ap = cen_d[:]
        nc.gpsimd.dma_start(
            out=cen,
            in_=bass.AP(
                tensor=cen_ap.tensor,
                offset=cen_ap.offset,
                ap=[[0, 128]] + list(cen_ap.ap),
            ),
        )
        bias_hpi = consts.tile([128, 1], F32)
        nc.vector.memset(bias_hpi, math.pi / 2.0)
        bias_lnp = consts.tile([128, 1], F32)
        nc.vector.memset(bias_lnp, -math.log(P_BASIS))

        # ---------------- distances: e = d / P ----------------
        # separate x / y streams (unit stride -> 2x TS modes)
        xf = consts.tile([128, NPP], F32)
        yf = consts.tile([128, NPP], F32)
        xy_r = xy_d[:, :].rearrange("(p j) c -> p j c", p=128)
        nc.sync.dma_start(out=xf, in_=xy_r[:, :, 0])
        nc.sync.dma_start(out=yf, in_=xy_r[:, :, 1])

        dx = consts.tile([128, NPP], F32)
        dy = consts.tile([128, NPP], F32)
        nc.vector.tensor_scalar(dx, xf, cen[:, 0:1], None, OP.subtract)
        nc.vector.tensor_scalar(dy, yf, cen[:, 1:2], None, OP.subtract)
        d2 = consts.tile([128, NPP], F32)
        dy2 = consts.tile([128, NPP], F32)
        nc.vector.tensor_tensor(d2, dx, dx, OP.mult)
        nc.vector.tensor_tensor(dy2, dy, dy, OP.mult)
        nc.vector.tensor_tensor(d2, d2, dy2, OP.add)
        nc.vector.tensor_scalar(d2, d2, 1e-12, None, OP.max)
        # e = d/P = exp(0.5*ln(d2) - ln(P))
        ld = consts.tile([128, NPP], F32)
        nc.scalar.activation(out=ld, in_=d2, func=AF.Ln)
        ev = consts.tile([128, NPP], F32)
        nc.scalar.activation(out=ev, in_=ld, func=AF.Exp, scale=0.5,
                             bias=bias_lnp[:, :])

        # ---------------- main loop ----------------
        psM = psump.tile([128, K_B], F32)  # M[w, k] accumulator
        tid_r = tid_d[:, :].rearrange("(p j) w -> p j w", p=128)
        n_super = (NPP + NT - 1) // NT
        for si in range(n_super):
            j0 = si * NT
            nt = min(NT, NPP - j0)
            tid16 = tid16p.tile([128, NT, W], F16, tag="tid16")
            if _TID_LOAD == "swdge":
                nc.gpsimd.dma_start(out=tid16[:, :nt, :], in_=tid_r[:, j0:j0 + nt, :])
            else:
                tid32 = tid32p.tile([128, NT, W], F32, tag="tid32")
                nc.sync.dma_start(out=tid32[:, :nt, :], in_=tid_r[:, j0:j0 + nt, :])
                if _TID_LOAD == "dve":
                    nc.vector.tensor_copy(tid16[:, :nt, :], tid32[:, :nt, :])
                else:  # split DVE / ScalarE
                    h = nt // 2
                    nc.vector.tensor_copy(tid16[:, :h, :], tid32[:, :h, :])
                    nc.scalar.copy(tid16[:, h:nt, :], tid32[:, h:nt, :])

            # u[p, t, k] = e[p, j0+t] * k  via stride-0-broadcast TT
            u_t = up.tile([128, NT, KH], F32, tag="u")
            e_sl = ev[:, j0:j0 + nt]
            e_b = bass.AP(
                tensor=e_sl.tensor,
                offset=e_sl.offset,
                ap=[list(e_sl.ap[0]), list(e_sl.ap[1]), [0, KH]],
            )
            k_b = bass.AP(
                tensor=kb.tensor,
                offset=kb.offset,
                ap=[list(kb.ap[0]), [0, nt], list(kb.ap[1])],
            )
            nc.vector.tensor_tensor(u_t[:, :nt, :], e_b, k_b, OP.mult)

            # m = round(u); ds = u - m; ads = |ds|
            m_t = mp.tile([128, NT, KH], F32, tag="m")
            nc.vector.tensor_scalar(
                m_t[:, :nt, :], u_t[:, :nt, :], MAGIC, MAGIC, OP.add, OP.subtract
            )
            ds_t = dsp.tile([128, NT, KH], F32, tag="ds")
            nc.vector.tensor_tensor(
                ds_t[:, :nt, :], u_t[:, :nt, :], m_t[:, :nt, :], OP.subtract
            )
            ads_t = adsp.tile([128, NT, KH], F32, tag="ads")
            nc.vector.tensor_scalar(
                ads_t[:, :nt, :].bitcast(U32),
                ds_t[:, :nt, :].bitcast(U32),
                0x7FFFFFFF,
                None,
                OP.bitwise_and,
            )

            # phi tile: [ones | cos_k | sin_k]
            trig = trigp.tile([128, NT, K_B], F16, tag="trig")
            nc.gpsimd.memset(trig[:, :nt, 0:1], 1.0)
            nc.scalar.activation(
                out=trig[:, :nt, 1:1 + KH],
                in_=ads_t[:, :nt, :],
                func=AF.Sin,
                bias=bias_hpi[:, :],
                scale=-TWO_PI,
            )
            nc.scalar.activation(
                out=trig[:, :nt, 1 + KH:K_B],
                in_=ds_t[:, :nt, :],
                func=AF.Sin,
                scale=TWO_PI,
            )

            for t in range(nt):
                j = j0 + t
                nc.tensor.matmul(
                    psM[:, :],
                    lhsT=tid16[:, t, :],
                    rhs=trig[:, t, :],
                    start=(j == 0),
                    stop=(j == NPP - 1),
                )

        # ---------------- epilogue ----------------
        msb = consts.tile([128, K_B], F32)
        nc.vector.tensor_copy(msb, psM)
        nc.sync.dma_start(out=out_d[:, :], in_=msb)

    nc.compile()
    return nc


def _get_program():
    global _cached_nc
    if _cached_nc is None:
        _cached_nc = _build_program()
    return _cached_nc


# ---------------- host-side basis fit ----------------
_FIT_CACHE = None


def _fit_matrix():
    """Precompute pinv-style solve operator for the ridge LS fit."""
    global _FIT_CACHE
    if _FIT_CACHE is None:
        S = 3072
        dg = np.linspace(0.0, DMAX_FIT, S)
        k = np.arange(1, KH + 1)
        Phi = np.concatenate(
            [
                np.ones((S, 1)),
                np.cos(TWO_PI * np.outer(dg, k) / P_BASIS),
                np.sin(TWO_PI * np.outer(dg, k) / P_BASIS),
            ],
            axis=1,
        )  # [S, K_B] in device column order
        G = Phi.T @ Phi + (1e-8 * S) * np.eye(K_B)
        _FIT_CACHE = (np.linalg.solve(G, Phi.T), dg)
    return _FIT_CACHE


def _host_exact(xy, tid, center, wavelength):
    """Exact (slow) fallback for out-of-envelope inputs."""
    d = np.linalg.norm(xy.astype(np.float64) - center[None, :], axis=1)
    C = np.zeros(W); S = np.zeros(W)
    alpha = TWO_PI / wavelength.astype(np.float64)
    for lo in range(0, xy.shape[0], 50000):
        hi = min(lo + 50000, xy.shape[0])
        ph = np.outer(d[lo:hi], alpha)
        t = tid[lo:hi].astype(np.float64)
        C += (np.cos(ph) * t).sum(axis=0)
        S += (np.sin(ph) * t).sum(axis=0)
    return C / xy.shape[0], S / xy.shape[0]


# results of the last device run (for test harnesses to inspect timing)
last_run_results = None


def kernel(xy, tid, center, wavelength):
    global last_run_results
    xy = np.ascontiguousarray(np.asarray(xy), dtype=np.float32)
    tid = np.ascontiguousarray(np.asarray(tid), dtype=np.float32)
    center = np.ascontiguousarray(np.asarray(center), dtype=np.float32)
    wavelength = np.ascontiguousarray(np.asarray(wavelength), dtype=np.float32)

    # envelope check: corners of [0,1]^2 bound the max distance
    corners = np.array([[0, 0], [0, 1], [1, 0], [1, 1]], dtype=np.float64)
    dmax_rt = np.sqrt(((corners - center[None, :]) ** 2).sum(axis=1)).max()
    offsets = np.linspace(0.0, TWO_PI, N_OFFSETS)
    if wavelength.min() < LAM_MIN_OK or dmax_rt > DMAX_OK:
        C, S = _host_exact(xy, tid, center, wavelength)
        vals = C[:, None] * np.cos(offsets)[None, :] - S[:, None] * np.sin(offsets)[None, :]
        return np.float32(-vals.max(axis=1).sum())

    nc = _get_program()
    kvec = np.arange(1, KH + 1, dtype=np.float32)
    in_maps = []
    for c in range(N_CORES):
        lo = c * PER_CORE
        hi = lo + PER_CORE
        xp = np.zeros((N_PAD, 2), dtype=np.float32)
        xp[:PER_CORE] = xy[lo:hi]
        tp = np.zeros((N_PAD, W), dtype=np.float32)
        tp[:PER_CORE] = tid[lo:hi]
        in_maps.append({"xy": xp, "tid": tp, "center": center, "kvec": kvec})

    res = run_bass_kernel_spmd(
        nc,
        in_maps,
        list(range(N_CORES)),
        trace=bool(int(os.environ.get("KERNEL_TRACE", "0"))),
    )
    last_run_results = res

    M = np.zeros((W, K_B), dtype=np.float64)
    for r in res.results:
        M += r["out"].astype(np.float64)

    # runtime wavelength fit: A[k, w] for cos targets, B for sin targets
    FIT, dg = _fit_matrix()
    alpha = TWO_PI / wavelength.astype(np.float64)
    A = FIT @ np.cos(np.outer(dg, alpha))  # [K_B, W]
    B = FIT @ np.sin(np.outer(dg, alpha))
    C = np.einsum("wk,kw->w", M, A) / N_POINTS
    S = np.einsum("wk,kw->w", M, B) / N_POINTS

    vals = C[:, None] * np.cos(offsets)[None, :] - S[:, None] * np.sin(offsets)[None, :]
    return np.float32(-vals.max(axis=1).sum())


# revision 5
# speedup vs baseline: 1.4733x; 1.4733x over previous
"""Trainium2 Bass kernel for nn_CenterModel (Fourier-basis formulation).

Computes -sum_w max_o ( C[w]*cos(o) - S[w]*sin(o) ) where
  C[w] = mean_n cos(2*pi*dist(n)/lambda[w]) * tid[n, w]
  S[w] = mean_n sin(2*pi*dist(n)/lambda[w]) * tid[n, w]

Key restructure vs the direct method: expand the wavelength-dependent
trig in a shared Fourier basis over distance,
  cos(alpha_w d) ~= sum_k A[k,w] phi_k(d),   phi = {1, cos(2*pi*k*d/P), sin(2*pi*k*d/P)}
so the device only computes K_B = 2*KH+1 = 45 basis columns per point
(instead of 128 wavelength columns) and one accumulated matmul
  M[w, k] = sum_n tid[n, w] * phi_k(d_n)        (PSUM, fp32)
The wavelength-dependent combine C = sum_k A[k,w] M[w,k] runs on the
host from a ridge least-squares fit against the *runtime* wavelengths
(fit residual < 2e-3 for lambda >= 0.048, d <= 0.71).

Device pipeline per 128-point tile group (NT=32 tiles):
  e = dist/P once per point (Ln/Exp sqrt, /P folded into Exp bias);
  u = e * k (one broadcast TT), m = round(u) (fused magic TS, 2x),
  ds = u - m (TT), ads = |ds| (TS bitand, 2x);
  sin cols = Sin(2*pi*ds), cos cols = Sin(pi/2 - 2*pi*ads) on ScalarE;
  tid cast fp32->fp16 split DVE/ScalarE; 32 matmuls tid^T @ phi.
8 cores shard the 500000 points; host sums the per-core M.
"""

import math
import os
from contextlib import ExitStack

import numpy as np

import concourse.bacc as bacc
import concourse.bass as bass
import concourse.tile as tile
from concourse import mybir
from concourse.bass_utils import run_bass_kernel_spmd

F32 = mybir.dt.float32
F16 = mybir.dt.float16
U32 = mybir.dt.uint32
AF = mybir.ActivationFunctionType
OP = mybir.AluOpType

N_POINTS = 500000
W = 128
N_OFFSETS = 50
N_CORES = 8
PER_CORE = N_POINTS // N_CORES  # 62500
NPP = 489                       # point-tiles per core
N_PAD = NPP * 128               # 62592 padded rows per core
NT = 32                         # point-tiles per super-tile
TWO_PI = 2.0 * math.pi

MAGIC = 12582912.0  # 1.5*2**23: fl(u+MAGIC)-MAGIC == round(u) for |u| < 2**22

# ---- Fourier basis config (must match host fit exactly) ----
P_BASIS = 0.8          # extension period
KH = 22                # harmonics 1..KH
K_B = 2 * KH + 1       # ones + cos_k + sin_k = 45
DMAX_FIT = math.sqrt(0.5) + 2e-3
# envelope in which the fit is trusted (runtime inputs checked on host)
LAM_MIN_OK = 0.048
DMAX_OK = math.sqrt(0.5) + 1e-6

_TID_LOAD = os.environ.get("KERNEL_TID_LOAD", "split")  # split | dve | swdge

_cached_nc = None


def _build_program():
    nc = bacc.Bacc(
        "TRN2",
        debug=False,
        enable_asserts=False,
        target_bir_lowering=False,
        num_devices=N_CORES,
    )
    xy_d = nc.dram_tensor("xy", [N_PAD, 2], F32, kind="ExternalInput")
    tid_d = nc.dram_tensor("tid", [N_PAD, W], F32, kind="ExternalInput")
    cen_d = nc.dram_tensor("center", [2], F32, kind="ExternalInput")
    kvec_d = nc.dram_tensor("kvec", [KH], F32, kind="ExternalInput")
    out_d = nc.dram_tensor("out", [K_B, W], F32, kind="ExternalOutput")

    with tile.TileContext(nc) as tc, ExitStack() as ctx:
        consts = ctx.enter_context(tc.tile_pool(name="consts", bufs=1))
        tid32p = ctx.enter_context(tc.tile_pool(name="tid32p", bufs=3))
        tid16p = ctx.enter_context(tc.tile_pool(name="tid16p", bufs=2))
        up = ctx.enter_context(tc.tile_pool(name="up", bufs=2))
        mp = ctx.enter_context(tc.tile_pool(name="mp", bufs=2))
        dsp = ctx.enter_context(tc.tile_pool(name="dsp", bufs=2))
        adsp = ctx.enter_context(tc.tile_pool(name="adsp", bufs=2))
        trigp = ctx.enter_context(tc.tile_pool(name="trigp", bufs=2))
        psump = ctx.enter_context(tc.tile_pool(name="psump", bufs=1, space="PSUM"))

        # ---------------- constants ----------------
        # kvec (harmonic indices 1..KH) broadcast to all partitions
        kb = consts.tile([128, KH], F32)
        kv_ap = kvec_d[:]
        nc.gpsimd.dma_start(
            out=kb,
            in_=bass.AP(
                tensor=kv_ap.tensor,
                offset=kv_ap.offset,
                ap=[[0, 128]] + list(kv_ap.ap),
            ),
        )
        # center broadcast to all partitions
        cen = consts.tile([128, 2], F32)
        cen_ap = cen_d[:]
        nc.gpsimd.dma_start(
            out=cen,
            in_=bass.AP(
                tensor=cen_ap.tensor,
                offset=cen_ap.offset,
                ap=[[0, 128]] + list(cen_ap.ap),
            ),
        )
        bias_hpi = consts.tile([128, 1], F32)
        nc.vector.memset(bias_hpi, math.pi / 2.0)
        bias_lnp = consts.tile([128, 1], F32)
        nc.vector.memset(bias_lnp, -math.log(P_BASIS))

        # ---------------- distances: e = d / P ----------------
        # xy loaded contiguously (one 3.9KB descriptor per partition);
        # the x/y split happens on-chip via stride-2 reads
        xyf = consts.tile([128, NPP, 2], F32)
        nc.sync.dma_start(out=xyf, in_=xy_d[:, :].rearrange("(p j) c -> p j c", p=128))

        dx = consts.tile([128, NPP], F32)
        dy = consts.tile([128, NPP], F32)
        nc.vector.tensor_scalar(dx, xyf[:, :, 0], cen[:, 0:1], None, OP.subtract)
        nc.vector.tensor_scalar(dy, xyf[:, :, 1], cen[:, 1:2], None, OP.subtract)
        d2 = consts.tile([128, NPP], F32)
        dy2 = consts.tile([128, NPP], F32)
        nc.vector.tensor_tensor(d2, dx, dx, OP.mult)
        nc.vector.tensor_tensor(dy2, dy, dy, OP.mult)
        nc.vector.tensor_tensor(d2, d2, dy2, OP.add)
        nc.vector.tensor_scalar(d2, d2, 1e-12, None, OP.max)
        # e = d/P = exp(0.5*ln(d2) - ln(P))
        ld = consts.tile([128, NPP], F32)
        nc.scalar.activation(out=ld, in_=d2, func=AF.Ln)
        ev = consts.tile([128, NPP], F32)
        nc.scalar.activation(out=ev, in_=ld, func=AF.Exp, scale=0.5,
                             bias=bias_lnp[:, :])

        # ---------------- main loop ----------------
        psM = psump.tile([K_B, W], F32)  # M[k, w] accumulator
        tid_r = tid_d[:, :].rearrange("(p j) w -> p j w", p=128)
        n_super = (NPP + NT - 1) // NT
        for si in range(n_super):
            j0 = si * NT
            nt = min(NT, NPP - j0)
            tid16 = tid16p.tile([128, NT, W], F16, tag="tid16")
            if _TID_LOAD == "swdge":
                nc.gpsimd.dma_start(out=tid16[:, :nt, :], in_=tid_r[:, j0:j0 + nt, :])
            else:
                tid32 = tid32p.tile([128, NT, W], F32, tag="tid32")
                nc.sync.dma_start(out=tid32[:, :nt, :], in_=tid_r[:, j0:j0 + nt, :])
                if _TID_LOAD == "dve":
                    nc.vector.tensor_copy(tid16[:, :nt, :], tid32[:, :nt, :])
                else:  # split DVE / ScalarE
                    h = nt // 2
                    nc.vector.tensor_copy(tid16[:, :h, :], tid32[:, :h, :])
                    nc.scalar.copy(tid16[:, h:nt, :], tid32[:, h:nt, :])

            # u[p, t, k] = e[p, j0+t] * k  via stride-0-broadcast TT
            u_t = up.tile([128, NT, KH], F32, tag="u")
            e_sl = ev[:, j0:j0 + nt]
            e_b = bass.AP(
                tensor=e_sl.tensor,
                offset=e_sl.offset,
                ap=[list(e_sl.ap[0]), list(e_sl.ap[1]), [0, KH]],
            )
            k_b = bass.AP(
                tensor=kb.tensor,
                offset=kb.offset,
                ap=[list(kb.ap[0]), [0, nt], list(kb.ap[1])],
            )
            nc.vector.tensor_tensor(u_t[:, :nt, :], e_b, k_b, OP.mult)

            # m = round(u); ds = u - m; ads = |ds|
            m_t = mp.tile([128, NT, KH], F32, tag="m")
            nc.vector.tensor_scalar(
                m_t[:, :nt, :], u_t[:, :nt, :], MAGIC, MAGIC, OP.add, OP.subtract
            )
            ds_t = dsp.tile([128, NT, KH], F32, tag="ds")
            nc.vector.tensor_tensor(
                ds_t[:, :nt, :], u_t[:, :nt, :], m_t[:, :nt, :], OP.subtract
            )
            ads_t = adsp.tile([128, NT, KH], F32, tag="ads")
            nc.vector.tensor_scalar(
                ads_t[:, :nt, :].bitcast(U32),
                ds_t[:, :nt, :].bitcast(U32),
                0x7FFFFFFF,
                None,
                OP.bitwise_and,
            )

            # phi tile: [ones | cos_k | sin_k]
            trig = trigp.tile([128, NT, K_B], F16, tag="trig")
            nc.gpsimd.memset(trig[:, :nt, 0:1], 1.0)
            nc.scalar.activation(
                out=trig[:, :nt, 1:1 + KH],
                in_=ads_t[:, :nt, :],
                func=AF.Sin,
                bias=bias_hpi[:, :],
                scale=-TWO_PI,
            )
            nc.scalar.activation(
                out=trig[:, :nt, 1 + KH:K_B],
                in_=ds_t[:, :nt, :],
                func=AF.Sin,
                scale=TWO_PI,
            )

            # trig stationary (45-col LDWEIGHTS = 38ns), tid moving
            # -> psM[k, w] = sum_n phi_k(d_n) tid[n, w]
            for t in range(nt):
                j = j0 + t
                nc.tensor.matmul(
                    psM[:, :],
                    lhsT=trig[:, t, :],
                    rhs=tid16[:, t, :],
                    start=(j == 0),
                    stop=(j == NPP - 1),
                )

        # ---------------- epilogue ----------------
        msb = consts.tile([K_B, W], F32)
        nc.vector.tensor_copy(msb, psM)
        nc.sync.dma_start(out=out_d[:, :], in_=msb)

    nc.compile()
    return nc


def _get_program():
    global _cached_nc
    if _cached_nc is None:
        _cached_nc = _build_program()
    return _cached_nc


# ---------------- host-side basis fit ----------------
_FIT_CACHE = None


def _fit_matrix():
    """Precompute pinv-style solve operator for the ridge LS fit."""
    global _FIT_CACHE
    if _FIT_CACHE is None:
        S = 3072
        dg = np.linspace(0.0, DMAX_FIT, S)
        k = np.arange(1, KH + 1)
        Phi = np.concatenate(
            [
                np.ones((S, 1)),
                np.cos(TWO_PI * np.outer(dg, k) / P_BASIS),
                np.sin(TWO_PI * np.outer(dg, k) / P_BASIS),
            ],
            axis=1,
        )  # [S, K_B] in device column order
        G = Phi.T @ Phi + (1e-8 * S) * np.eye(K_B)
        _FIT_CACHE = (np.linalg.solve(G, Phi.T), dg)
    return _FIT_CACHE


def _host_exact(xy, tid, center, wavelength):
    """Exact (slow) fallback for out-of-envelope inputs."""
    d = np.linalg.norm(xy.astype(np.float64) - center[None, :], axis=1)
    C = np.zeros(W); S = np.zeros(W)
    alpha = TWO_PI / wavelength.astype(np.float64)
    for lo in range(0, xy.shape[0], 50000):
        hi = min(lo + 50000, xy.shape[0])
        ph = np.outer(d[lo:hi], alpha)
        t = tid[lo:hi].astype(np.float64)
        C += (np.cos(ph) * t).sum(axis=0)
        S += (np.sin(ph) * t).sum(axis=0)
    return C / xy.shape[0], S / xy.shape[0]


# results of the last device run (for test harnesses to inspect timing)
last_run_results = None


def kernel(xy, tid, center, wavelength):
    global last_run_results
    xy = np.ascontiguousarray(np.asarray(xy), dtype=np.float32)
    tid = np.ascontiguousarray(np.asarray(tid), dtype=np.float32)
    center = np.ascontiguousarray(np.asarray(center), dtype=np.float32)
    wavelength = np.ascontiguousarray(np.asarray(wavelength), dtype=np.float32)

    # envelope check: corners of [0,1]^2 bound the max distance
    corners = np.array([[0, 0], [0, 1], [1, 0], [1, 1]], dtype=np.float64)
    dmax_rt = np.sqrt(((corners - center[None, :]) ** 2).sum(axis=1)).max()
    offsets = np.linspace(0.0, TWO_PI, N_OFFSETS)
    if wavelength.min() < LAM_MIN_OK or dmax_rt > DMAX_OK:
        C, S = _host_exact(xy, tid, center, wavelength)
        vals = C[:, None] * np.cos(offsets)[None, :] - S[:, None] * np.sin(offsets)[None, :]
        return np.float32(-vals.max(axis=1).sum())

    nc = _get_program()
    kvec = np.arange(1, KH + 1, dtype=np.float32)
    in_maps = []
    for c in range(N_CORES):
        lo = c * PER_CORE
        hi = lo + PER_CORE
        xp = np.zeros((N_PAD, 2), dtype=np.float32)
        xp[:PER_CORE] = xy[lo:hi]
        tp = np.zeros((N_PAD, W), dtype=np.float32)
        tp[:PER_CORE] = tid[lo:hi]
        in_maps.append({"xy": xp, "tid": tp, "center": center, "kvec": kvec})

    res = run_bass_kernel_spmd(
        nc,
        in_maps,
        list(range(N_CORES)),
        trace=bool(int(os.environ.get("KERNEL_TRACE", "0"))),
    )
    last_run_results = res

    M = np.zeros((K_B, W), dtype=np.float64)
    for r in res.results:
        M += r["out"].astype(np.float64)

    # runtime wavelength fit: A[k, w] for cos targets, B for sin targets
    FIT, dg = _fit_matrix()
    alpha = TWO_PI / wavelength.astype(np.float64)
    A = FIT @ np.cos(np.outer(dg, alpha))  # [K_B, W]
    B = FIT @ np.sin(np.outer(dg, alpha))
    C = np.einsum("kw,kw->w", M, A) / N_POINTS
    S = np.einsum("kw,kw->w", M, B) / N_POINTS

    vals = C[:, None] * np.cos(offsets)[None, :] - S[:, None] * np.sin(offsets)[None, :]
    return np.float32(-vals.max(axis=1).sum())
